# revision 3
# baseline (speedup 1.0000x reference)
"""Trainium2 Bass kernel for nn_Attention_87892210745803.

Full problem: x [4, 2048, 1024] fp32 -> fused QKV projection (W_qkv [3072, 1024],
b_qkv [3072]) -> 16-head causal attention (head size 64) -> out [4, 2048, 1024].

Sharding (8 cores): core c handles batch b = c // 2 and head-group g = c % 2
(8 of the 16 heads). Each core gets x[b] plus the W/b rows for its heads
(q | k | v blocks of 512 rows each) and produces out[b, :, g*512:(g+1)*512].

Per-core kernel (Bass/Tile, fp32 data, float32r matmuls):
  phase 1: transpose x and W tiles via PE (fp32 has no DMA transpose), compute
           q^T/k^T with f-on-partitions (ready for attention) and v in natural
           layout, biases folded appropriately.
  phase 2: per (i-block, head): s^T = k^T q (PE), exp on ACT, causal zeroing via
           gpsimd affine_select, o'^T accumulation with a ones-column folded into
           v to get the softmax denominator for free, PE transpose back, DVE
           divide, v-bias added at the end.
"""

import sys

sys.path.insert(0, "/opt/trn_rl_repo")

import numpy as np

B, T, E = 4, 2048, 1024
NH_GLOBAL = 16
HS = 64
P = 128
N_CORES = 8
H = 8  # heads per core
F = H * HS  # 512: rows per q/k/v block per core

_CACHE = {}


def _build_nc(T=T, E=E, H=H, IB=512, use_f32r=True):
    import concourse.bacc as bacc
    import concourse.mybir as mybir
    import concourse.tile as tile
    from concourse.masks import make_identity

    F32 = mybir.dt.float32
    OP_DT = mybir.dt.float32r if use_f32r else mybir.dt.float32
    F = H * HS
    EO = E // P  # contraction subtiles for QKV
    TT = T // P  # t-tiles
    FQK = 2 * F // P  # f-tiles for q+k
    FV_OFF = 2 * F  # v rows start in w_c
    TBS = min(512, T)  # t-block size for qkv rhs
    NTB = T // TBS
    NI = T // IB  # i-blocks per head
    JPI = IB // P  # j-tiles per i-block

    nc = bacc.Bacc("TRN2", target_bir_lowering=False, debug=False)
    x_d = nc.dram_tensor("x", [T, E], F32, kind="ExternalInput").ap()
    w_d = nc.dram_tensor("w", [3 * F, E], F32, kind="ExternalInput").ap()
    b_d = nc.dram_tensor("b", [3 * F], F32, kind="ExternalInput").ap()
    out_d = nc.dram_tensor("out", [T, F], F32, kind="ExternalOutput").ap()

    def mm(psum, lhsT, rhs, start, stop):
        nc.tensor.matmul(psum, lhsT, rhs, start=start, stop=stop)

    with tile.TileContext(nc) as tc:
        with (
            tc.tile_pool(name="const", bufs=1) as const_pool,
            tc.tile_pool(name="persist", bufs=1) as persist,
        ):
            identity = const_pool.tile([P, P], F32)
            make_identity(nc, identity)
            b_sb = const_pool.tile([P, 3 * F // P], F32)
            nc.sync.dma_start(b_sb[:], b_d.rearrange("(o p) -> p o", p=P))
            bias_v = const_pool.tile([P, F], F32)
            nc.sync.dma_start(
                bias_v[:], b_d[None, FV_OFF : FV_OFF + F].to_broadcast((P, F))
            )

            qkT = persist.tile([P, FQK, T], OP_DT)
            v_aug = persist.tile([P, TT, H, HS + 1], OP_DT)
            ones_col = const_pool.tile([P, 1], F32)
            nc.vector.memset(ones_col, 1.0)
            # fp32r tiles need rounding producers; a converting copy qualifies
            nc.vector.tensor_copy(
                v_aug[:, :, :, HS : HS + 1],
                ones_col[:, None, None, :].to_broadcast((P, TT, H, 1)),
            )

            # ================= phase 1: QKV projection =================
            with (
                tc.tile_pool(name="wT", bufs=1) as wT_pool,
                tc.tile_pool(name="stage", bufs=2) as stage,
                tc.tile_pool(name="xT", bufs=2) as xT_pool,
                tc.tile_pool(name="trps", bufs=2, space="PSUM") as trps,
                tc.tile_pool(name="mmps", bufs=2, space="PSUM") as mmps,
            ):
                wT = wT_pool.tile([P, EO, 3 * F], OP_DT)
                for wf in range(3 * F // P):
                    w_raw = stage.tile([P, E], F32, tag="stage", name="w_raw")
                    half = E // 2
                    nc.sync.dma_start(
                        w_raw[:, :half], w_d[wf * P : (wf + 1) * P, :half]
                    )
                    nc.sync.dma_start(
                        w_raw[:, half:], w_d[wf * P : (wf + 1) * P, half:]
                    )
                    for eo in range(EO):
                        tps = trps.tile([P, P], F32, name="tps")
                        nc.tensor.transpose(
                            tps, w_raw[:, eo * P : (eo + 1) * P], identity
                        )
                        nc.vector.tensor_copy(wT[:, eo, wf * P : (wf + 1) * P], tps)

                for tb in range(NTB):
                    xT = xT_pool.tile([P, EO, TBS], OP_DT, tag="xT", name="xT")
                    for tt in range(TBS // P):
                        git = tb * (TBS // P) + tt
                        x_raw = stage.tile([P, E], F32, tag="stage", name="x_raw")
                        half = E // 2
                        nc.sync.dma_start(
                            x_raw[:, :half], x_d[git * P : (git + 1) * P, :half]
                        )
                        nc.sync.dma_start(
                            x_raw[:, half:], x_d[git * P : (git + 1) * P, half:]
                        )
                        for eo in range(EO):
                            tps = trps.tile([P, P], F32, name="tps")
                            nc.tensor.transpose(
                                tps, x_raw[:, eo * P : (eo + 1) * P], identity
                            )
                            nc.vector.tensor_copy(
                                xT[:, eo, tt * P : (tt + 1) * P], tps
                            )

                    # q^T / k^T tiles: psum[f=128, t=512]
                    for wf in range(FQK):
                        ps = mmps.tile([P, 512], F32, tag="mmps", name="qkps")[
                            :, :TBS
                        ]
                        for eo in range(EO):
                            mm(
                                ps,
                                wT[:, eo, wf * P : (wf + 1) * P],
                                xT[:, eo, :],
                                start=(eo == 0),
                                stop=(eo == EO - 1),
                            )
                        nc.vector.tensor_scalar_add(
                            qkT[:, wf, tb * TBS : (tb + 1) * TBS],
                            ps,
                            b_sb[:, wf : wf + 1],
                        )

                    # v tiles: psum[t=128, f=512], bias folded in at the end
                    for tt in range(TBS // P):
                        git = tb * (TBS // P) + tt
                        ps = mmps.tile([P, 512], F32, tag="mmps", name="vps")[:, :F]
                        for eo in range(EO):
                            mm(
                                ps,
                                xT[:, eo, tt * P : (tt + 1) * P],
                                wT[:, eo, FV_OFF : FV_OFF + F],
                                start=(eo == 0),
                                stop=(eo == EO - 1),
                            )
                        nc.vector.tensor_copy(
                            v_aug[:, git, :, 0:HS],
                            ps.rearrange("p (h d) -> p h d", d=HS),
                        )

            # ================= phase 2: attention =================
            with (
                tc.tile_pool(name="exp", bufs=4) as exp_pool,
                tc.tile_pool(name="oT", bufs=2) as oT_pool,
                tc.tile_pool(name="recip", bufs=4) as recip_pool,
                tc.tile_pool(name="outsb", bufs=2) as out_pool,
                tc.tile_pool(name="sps", bufs=2, space="PSUM") as sps,
                tc.tile_pool(name="ops", bufs=2, space="PSUM") as ops,
                tc.tile_pool(name="tps2", bufs=2, space="PSUM") as tps2,
            ):
                for I in range(NI):
                    out_sb = out_pool.tile([P, IB // P, F], F32, tag="outsb")
                    njt = JPI * (I + 1)
                    for h in range(H):
                        pb = (h % 2) * HS
                        fq = h // 2
                        fk = F // P + h // 2
                        ops_t = ops.tile([P, 512], F32, tag="ops", name="ops_t")[
                            :, :IB
                        ]
                        for jt in range(njt):
                            sp = sps.tile([P, 512], F32, tag="sps", name="sp")[
                                :, :IB
                            ]
                            mm(
                                sp,
                                qkT[pb : pb + HS, fk, jt * P : (jt + 1) * P],
                                qkT[pb : pb + HS, fq, I * IB : (I + 1) * IB],
                                start=True,
                                stop=True,
                            )
                            ex = exp_pool.tile([P, IB], OP_DT, tag="exp")
                            nc.scalar.activation(
                                ex, sp, mybir.ActivationFunctionType.Exp, scale=0.125
                            )
                            r = jt - JPI * I
                            if r >= 0:
                                # causal: keep where i >= j  <=>  y - p - P*r >= 0
                                nc.gpsimd.affine_select(
                                    out=ex,
                                    in_=ex,
                                    compare_op=mybir.AluOpType.is_ge,
                                    fill=0.0,
                                    base=-P * r,
                                    channel_multiplier=-1,
                                    pattern=[[1, IB]],
                                )
                            mm(
                                ops_t[: HS + 1, :],
                                v_aug[:, jt, h, :],
                                ex,
                                start=(jt == 0),
                                stop=(jt == njt - 1),
                            )
                        oT = oT_pool.tile([P, IB], F32, tag="oT")
                        nc.vector.tensor_copy(oT[: HS + 1, :], ops_t[: HS + 1, :])
                        for it in range(IB // P):
                            tp = tps2.tile([P, P], F32, tag="tps2", name="tp")
                            nc.tensor.transpose(
                                tp[:, : HS + 1],
                                oT[: HS + 1, it * P : (it + 1) * P],
                                identity[: HS + 1, : HS + 1],
                            )
                            rc = recip_pool.tile([P, 1], F32, tag="recip", name="rc")
                            nc.vector.reciprocal(rc, tp[:, HS : HS + 1])
                            nc.vector.tensor_scalar_mul(
                                out_sb[:, it, h * HS : (h + 1) * HS],
                                tp[:, 0:HS],
                                rc,
                            )
                    for it in range(IB // P):
                        git = I * (IB // P) + it
                        nc.vector.tensor_add(
                            out=out_sb[:, it, :], in0=out_sb[:, it, :], in1=bias_v
                        )
                        nc.sync.dma_start(
                            out_d[git * P : (git + 1) * P, :], out_sb[:, it, :]
                        )

    nc.compile()
    return nc


def get_nc():
    if "nc" not in _CACHE:
        _CACHE["nc"] = _build_nc()
    return _CACHE["nc"]


def shard_inputs(x, W_qkv, b_qkv):
    """Split full inputs into the 8 per-core input maps."""
    in_maps = []
    for c in range(N_CORES):
        b_, g = c // 2, c % 2
        rq = slice(g * F, (g + 1) * F)
        rk = slice(E + g * F, E + (g + 1) * F)
        rv = slice(2 * E + g * F, 2 * E + (g + 1) * F)
        w_c = np.concatenate([W_qkv[rq], W_qkv[rk], W_qkv[rv]], axis=0)
        b_c = np.concatenate([b_qkv[rq], b_qkv[rk], b_qkv[rv]], axis=0)
        in_maps.append(
            {
                "x": np.ascontiguousarray(x[b_], dtype=np.float32),
                "w": np.ascontiguousarray(w_c, dtype=np.float32),
                "b": np.ascontiguousarray(b_c, dtype=np.float32),
            }
        )
    return in_maps


def gather_output(results):
    """Assemble per-core [T, F] outputs into the full [B, T, E] output."""
    out = np.empty((B, T, E), dtype=np.float32)
    for c in range(N_CORES):
        b_, g = c // 2, c % 2
        out[b_, :, g * F : (g + 1) * F] = results[c]["out"]
    return out


def kernel(x, W_qkv, b_qkv):
    from concourse.bass_utils import run_bass_kernel_spmd

    x = np.asarray(x, dtype=np.float32)
    W_qkv = np.asarray(W_qkv, dtype=np.float32)
    b_qkv = np.asarray(b_qkv, dtype=np.float32)
    in_maps = shard_inputs(x, W_qkv, b_qkv)
    res = run_bass_kernel_spmd(get_nc(), in_maps, core_ids=list(range(N_CORES)))
    return gather_output(res.results)
